# revision 26
# baseline (speedup 1.0000x reference)
"""FWHT kernel for Trainium2: y = FWHT(x) along last axis.

x: (8192, 4096) fp32. Sharded row-wise (data-parallel) across 8 NeuronCores.

Math: FWHT (natural order) is y[i] = sum_j (-1)^{<i,j>} x[j] over 12-bit
indices, which factorizes over any aligned bit split. Split j = (j1h:5 |
j1m:2 | j0:5) and i likewise:
  y[i1h,i1m,i0] = sum H32[j1h,i1h] H4[j1m,i1m] H32[j0,i0] x[j1h,j1m,j0]

Layout is chosen so every DMA descriptor is a 512B contiguous run (the DMA
cost model charges 2x below 512B). Rows are interleaved 4-way into
partition groups: partition p = rg*32 + k, with rg = row mod 4 inside a
16-row tile (row_local = 4r + rg, r in [0,4)). The HBM access pattern
[(128,128),(16384,4),(1,128)] then has 128-element (512B) contiguous runs.

Per 16-row tile (free dim 512):
  SP     : DMA load  X[p=(rg,j1h), f=(r,j1m,j0)] as f32r    (512B runs)
  TENSOR : MM1  Z = B^T X, B = I4 (x) H32, contract j1h -> PSUM zz (f32r;
           DRAM x is declared f32r so the BIR verifier accepts DMA->matmul)
  VECTOR : T1   32x32 block transpose zz -> tq[p=(rg,j0)] SBUF f32
  POOL   : convert tq f32 -> tt bf16 (GPSIMD is the rounding producer for
           MM2's inputs; it cannot read PSUM, hence the tq staging)
  TENSOR : MM2  16 matmuls: W[.,(r,i1m,i1h)] += H4[j1m,i1m]*(B^T T) (bf16)
  VECTOR : T2   block transpose -> O[p=(rg,i1h), f=(r,i1m,i0)] SBUF f32
  ACT    : DMA store O -> y                                  (512B runs)

Scheduling: all DMAs move tile PAIRS (32 rows, 1024 descriptors) to halve
pressure on the shared HWDGE slot; T1/T2/conv are also pair-batched to
amortize access latency.  Waits are ATTACHED to instructions (one slot per
instruction, resolved in the engine wait queue) so they never block the
SEQ; only slack slot-recycling waits are standalone.  The critical cycle
T1 -> conv -> MM2 -> T2 -> (DVE in-order) -> T1 spans ~6.5us, so T1 runs
3 pairs ahead of T2 in the DVE stream (and MM1 6 tiles ahead on PE),
bringing the per-pair cost under the 2912 ns DMA cadence.  Weight loads
go through Pool/SWDGE so they never contend with data loads for HWDGE.

Per-core DMA: 32 MB at 512B/desc = 93.5 us, and the DMA device simulates
100% busy end-to-end; per-pair engine busy: DMA 2912 ns > DVE ~2380 >
PE ~2130 > Pool ~1520 > ACT ~690.  TimelineSim: 96780 ns/core vs the
195236 ns baseline (which paid the 2x sub-512B descriptor penalty).

Precision: x in f32r (~1e-4 rel), T in bf16 (~4e-3), weights are +-1
(exact); PSUM accumulation f32. Well within the 2e-2 tolerance.
"""

import numpy as np

N_CORES = 8
ROWS = 8192
COLS = 4096
ROWS_PER_CORE = ROWS // N_CORES  # 1024
R_TILE = 16                      # rows per tile -> free dim 512
N_ITERS = ROWS_PER_CORE // R_TILE  # 64

B_IN = 16   # xin slots
B_MID = 8   # tq/tt slots (pair-aligned)
B_OUT = 12  # oo slots (pair-aligned)
N_PSUM = 4  # zz/ww slots (4 banks each)
LA = 6      # MM1 lookahead (tiles) over the MM2 stream


def _sylvester(n: int) -> np.ndarray:
    H = np.array([[1.0]], dtype=np.float32)
    while H.shape[0] < n:
        H = np.block([[H, H], [H, -H]])
    return H.astype(np.float32)


def _weights():
    import ml_dtypes

    B = np.kron(np.eye(4, dtype=np.float32), _sylvester(32)).astype(np.float32)
    Bb = np.concatenate([B, -B], axis=1).astype(ml_dtypes.bfloat16)
    return np.ascontiguousarray(B), np.ascontiguousarray(Bb)


def _build_nc(n_iters: int = N_ITERS):
    import concourse.bass as bass
    import concourse.mybir as mybir

    assert n_iters % 2 == 0
    f32 = mybir.dt.float32
    f32r = mybir.dt.float32r
    bf16 = mybir.dt.bfloat16

    # detect_race_conditions=False: waits use sum semantics (each DMA adds
    # exactly +16 split across SDMA engines, per-engine FIFO), so >= 16*k
    # implies the first k DMAs completed.
    nc = bass.Bass(detect_race_conditions=False)
    rows_total = n_iters * R_TILE
    x = nc.declare_dram_parameter("x", [rows_total, COLS], f32r, isOutput=False)
    bw_d = nc.declare_dram_parameter("bw", [128, 128], f32r, isOutput=False)
    bwb_d = nc.declare_dram_parameter("bwb", [128, 256], bf16, isOutput=False)
    y = nc.declare_dram_parameter("y", [rows_total, COLS], f32, isOutput=True)

    with (
        nc.sbuf_tensor("bw_sb", [128, 128], f32r) as bw,
        nc.sbuf_tensor("bwb_sb", [128, 256], bf16) as bwb,
        nc.sbuf_tensor("xin", [128, B_IN * 512], f32r) as xin,
        nc.sbuf_tensor("tq", [128, B_MID * 512], f32) as tq,
        nc.sbuf_tensor("tt", [128, B_MID * 512], bf16) as tt,
        nc.sbuf_tensor("oo", [128, B_OUT * 512], f32) as oo,
        nc.psum_tensor("zz", [128, N_PSUM * 512], f32) as zz,
        nc.psum_tensor("ww", [128, N_PSUM * 512], f32) as ww,
        nc.semaphore("load_sem") as load_sem,
        nc.semaphore("store_sem") as store_sem,
        nc.semaphore("pool_sem") as pool_sem,
        nc.semaphore("wt_sem") as wt_sem,
        nc.semaphore("pe1_sem") as pe1_sem,
        nc.semaphore("pe2_sem") as pe2_sem,
        nc.semaphore("dve1_sem") as dve1_sem,
        nc.semaphore("dve2_sem") as dve2_sem,
        nc.Block() as block,
    ):
        def slot(buf, i, n=N_PSUM):
            return buf[:, (i % n) * 512:(i % n + 1) * 512]

        def pair(buf, k, n=N_PSUM):
            # tiles (2k, 2k+1) -> contiguous [128, 1024] view
            return buf[:, (2 * k % n) * 512:(2 * k % n + 2) * 512]

        def sub32(ap, k):
            # [128, 512] slot -> [(p), (r: stride 128, 4), (c: 1, 32)] at
            # 32-column subblock k of each 128-run
            return ap.rearrange("p (r k c) -> p r k c", k=4, c=32)[:, :, k, :]

        @block.sync
        def _(sync):
            for q in range(n_iters // 2):
                src = x[2 * q * R_TILE:(2 * q + 2) * R_TILE, :].rearrange(
                    "(r rg) (ph inner) -> (rg ph) r inner", rg=4, inner=128
                )
                ld = sync.dma_start(
                    pair(xin, q, B_IN).rearrange(
                        "p (r inner) -> p r inner", inner=128
                    ),
                    src,
                )
                if 2 * q >= B_IN:
                    ld.wait_op(pe1_sem, 2 * q - B_IN + 2, "sem-ge")
                ld.then_inc(load_sem, 16)

        @block.tensor
        def _(tensor):
            tensor.wait_ge(wt_sem, 32)  # both weight DMAs done

            def mm1(i):
                if i >= N_PSUM:
                    # slack wait (T1 runs ahead): standalone, normally already
                    # satisfied so it doesn't hold the SEQ
                    tensor.wait_ge(dve1_sem, (i - N_PSUM) // 2 + 1)
                mm = tensor.matmul(
                    out=slot(zz, i),
                    lhsT=bw[:],
                    rhs=slot(xin, i, B_IN),
                    start=True,
                    stop=True,
                )
                mm.wait_op(load_sem, 16 * (i // 2 + 1), "sem-ge")
                mm.then_inc(pe1_sem)

            def mm2(j):
                if j >= N_PSUM:
                    tensor.wait_ge(dve2_sem, (j - N_PSUM) // 2 + 1)  # slack
                for i1m in range(4):
                    for j1m in range(4):
                        neg = bin(i1m & j1m).count("1") % 2
                        lhs = bwb[:, 128:256] if neg else bwb[:, 0:128]
                        mm = tensor.matmul(
                            out=sub32(slot(ww, j), i1m),
                            lhsT=lhs,
                            rhs=sub32(slot(tt, j, B_MID), j1m),
                            start=(j1m == 0),
                            stop=(j1m == 3),
                        )
                        if i1m == 0 and j1m == 0:
                            mm.wait_op(pool_sem, j // 2 + 1, "sem-ge")
                        if i1m == 3 and j1m == 3:
                            mm.then_inc(pe2_sem)

            for i in range(min(LA, n_iters)):
                mm1(i)
            for j in range(n_iters):
                if j + LA < n_iters:
                    mm1(j + LA)
                mm2(j)

        @block.gpsimd
        def _(gpsimd):
            # weight loads via SWDGE so they never contend with the first
            # data loads for the shared HWDGE slot
            gpsimd.dma_start(bw[:], bw_d[:]).then_inc(wt_sem, 16)
            gpsimd.dma_start(bwb[:], bwb_d[:]).then_inc(wt_sem, 16)
            # f32 -> bf16 rounding copy (SBUF->SBUF, pair-batched) on the
            # otherwise-idle Pool engine; GPSIMD cannot access PSUM, so T1
            # stages Z into tq first.
            for c in range(n_iters // 2):
                if 2 * c >= B_MID:
                    gpsimd.wait_ge(pe2_sem, 2 * c - B_MID + 2)  # tt free
                cp = gpsimd.tensor_copy(pair(tt, c, B_MID), pair(tq, c, B_MID))
                cp.wait_op(dve1_sem, c + 1, "sem-ge")
                cp.then_inc(pool_sem)

        @block.scalar
        def _(scalar):
            for k in range(n_iters // 2):
                dst = y[2 * k * R_TILE:(2 * k + 2) * R_TILE, :].rearrange(
                    "(r rg) (ph inner) -> (rg ph) r inner", rg=4, inner=128
                )
                st = scalar.dma_start(
                    dst,
                    pair(oo, k, B_OUT).rearrange(
                        "p (r inner) -> p r inner", inner=128
                    ),
                )
                st.wait_op(dve2_sem, k + 1, "sem-ge")
                st.then_inc(store_sem, 16)

        @block.vector
        def _(vector):
            n_pairs = n_iters // 2

            def t1(k):
                if 2 * k >= B_MID:
                    # tq slot free: conv of B_MID//2 pairs ago done
                    vector.wait_ge(pool_sem, k - B_MID // 2 + 1)
                tr = vector.transpose(pair(tq, k, B_MID), pair(zz, k))
                tr.wait_op(pe1_sem, 2 * k + 2, "sem-ge")
                tr.then_inc(dve1_sem)

            def t2(k):
                if 2 * k >= B_OUT:
                    vector.wait_ge(store_sem, 16 * (k - B_OUT // 2 + 1))  # slack
                tr = vector.transpose(pair(oo, k, B_OUT), pair(ww, k))
                tr.wait_op(pe2_sem, 2 * k + 2, "sem-ge")
                tr.then_inc(dve2_sem)

            for k in range(min(3, n_pairs)):
                t1(k)
            for k in range(n_pairs):
                if k + 3 < n_pairs:
                    t1(k + 3)
                t2(k)


    return nc


_CACHE = {}


def kernel(x: np.ndarray) -> np.ndarray:
    from concourse.bass_utils import run_bass_kernel_spmd

    assert x.shape == (ROWS, COLS) and x.dtype == np.float32

    if "nc" not in _CACHE:
        _CACHE["nc"] = _build_nc()
    nc = _CACHE["nc"]

    bw, bwb = _weights()

    core_ids = list(range(N_CORES))
    in_maps = [
        {
            "x": np.ascontiguousarray(x[i * ROWS_PER_CORE:(i + 1) * ROWS_PER_CORE]),
            "bw": bw,
            "bwb": bwb,
        }
        for i in core_ids
    ]
    res = run_bass_kernel_spmd(nc, in_maps, core_ids)
    out = np.empty((ROWS, COLS), dtype=np.float32)
    for i in core_ids:
        out[i * ROWS_PER_CORE:(i + 1) * ROWS_PER_CORE] = res.results[i]["y"]
    return out


# revision 29
# speedup vs baseline: 1.0026x; 1.0026x over previous
"""FWHT kernel for Trainium2: y = FWHT(x) along last axis.

x: (8192, 4096) fp32. Sharded row-wise (data-parallel) across 8 NeuronCores.

Math: FWHT (natural order) is y[i] = sum_j (-1)^{<i,j>} x[j] over 12-bit
indices, which factorizes over any aligned bit split. Split j = (j1h:5 |
j1m:2 | j0:5) and i likewise:
  y[i1h,i1m,i0] = sum H32[j1h,i1h] H4[j1m,i1m] H32[j0,i0] x[j1h,j1m,j0]

Layout is chosen so every DMA descriptor is a 512B contiguous run (the DMA
cost model charges 2x below 512B). Rows are interleaved 4-way into
partition groups: partition p = rg*32 + k, with rg = row mod 4 inside a
16-row tile (row_local = 4r + rg, r in [0,4)). The HBM access pattern
[(128,128),(16384,4),(1,128)] then has 128-element (512B) contiguous runs.

Per 16-row tile (free dim 512):
  SP     : DMA load  X[p=(rg,j1h), f=(r,j1m,j0)] as f32r    (512B runs)
  TENSOR : MM1  Z = B^T X, B = I4 (x) H32, contract j1h -> PSUM zz (f32r;
           DRAM x is declared f32r so the BIR verifier accepts DMA->matmul)
  VECTOR : T1   32x32 block transpose zz -> tq[p=(rg,j0)] SBUF f32
  POOL   : convert tq f32 -> tt bf16 (GPSIMD is the rounding producer for
           MM2's inputs; it cannot read PSUM, hence the tq staging)
  TENSOR : MM2  16 matmuls: W[.,(r,i1m,i1h)] += H4[j1m,i1m]*(B^T T) (bf16)
  VECTOR : T2   block transpose -> O[p=(rg,i1h), f=(r,i1m,i0)] SBUF f32
  ACT    : DMA store O -> y                                  (512B runs)

Scheduling: all DMAs move tile PAIRS (32 rows, 1024 descriptors) to halve
pressure on the shared HWDGE slot; T1/T2/conv are also pair-batched to
amortize access latency.  Waits are ATTACHED to instructions (one slot per
instruction, resolved in the engine wait queue) so they never block the
SEQ; only slack slot-recycling waits are standalone.  The critical cycle
T1 -> conv -> MM2 -> T2 -> (DVE in-order) -> T1 spans ~6.5us, so T1 runs
3 pairs ahead of T2 in the DVE stream (and MM1 6 tiles ahead on PE),
bringing the per-pair cost under the 2912 ns DMA cadence.  Weight loads
go through Pool/SWDGE so they never contend with data loads for HWDGE.

Per-core DMA: 32 MB at 512B/desc = 93.5 us, and the DMA device simulates
100% busy end-to-end; per-pair engine busy: DMA 2912 ns > DVE ~2380 >
PE ~2130 > Pool ~1520 > ACT ~690.  TimelineSim: 96780 ns/core vs the
195236 ns baseline (which paid the 2x sub-512B descriptor penalty).

Precision: x in f32r (~1e-4 rel), T in bf16 (~4e-3), weights are +-1
(exact); PSUM accumulation f32. Well within the 2e-2 tolerance.
"""

import numpy as np

N_CORES = 8
ROWS = 8192
COLS = 4096
ROWS_PER_CORE = ROWS // N_CORES  # 1024
R_TILE = 16                      # rows per tile -> free dim 512
N_ITERS = ROWS_PER_CORE // R_TILE  # 64

B_IN = 16   # xin slots
B_MID = 8   # tq/tt slots (pair-aligned)
B_OUT = 12  # oo slots (pair-aligned)
N_PSUM = 4  # zz/ww slots (4 banks each)
LA = 6      # MM1 lookahead (tiles) over the MM2 stream


def _sylvester(n: int) -> np.ndarray:
    H = np.array([[1.0]], dtype=np.float32)
    while H.shape[0] < n:
        H = np.block([[H, H], [H, -H]])
    return H.astype(np.float32)


def _weights():
    import ml_dtypes

    B = np.kron(np.eye(4, dtype=np.float32), _sylvester(32)).astype(np.float32)
    Bb = np.concatenate([B, -B], axis=1).astype(ml_dtypes.bfloat16)
    return np.ascontiguousarray(B), np.ascontiguousarray(Bb)


def _build_nc(n_iters: int = N_ITERS):
    import concourse.bass as bass
    import concourse.mybir as mybir

    assert n_iters % 2 == 0
    f32 = mybir.dt.float32
    f32r = mybir.dt.float32r
    bf16 = mybir.dt.bfloat16

    # Skip Bass.__init__'s const-AP Memsets on Pool: nothing in this kernel
    # reads the const tensors (no activation-bias users), and they make Pool
    # the laggard of the startup all_engine_barrier, delaying the first load
    # DMA by ~0.5us.
    _orig_memset = bass.BassGpSimd.memset
    bass.BassGpSimd.memset = lambda self, ap, value: None
    try:
        # detect_race_conditions=False: waits use sum semantics (each DMA adds
        # exactly +16 split across SDMA engines, per-engine FIFO), so >= 16*k
        # implies the first k DMAs completed.
        nc = bass.Bass(detect_race_conditions=False)
    finally:
        bass.BassGpSimd.memset = _orig_memset
    rows_total = n_iters * R_TILE
    x = nc.declare_dram_parameter("x", [rows_total, COLS], f32r, isOutput=False)
    bw_d = nc.declare_dram_parameter("bw", [128, 128], f32r, isOutput=False)
    bwb_d = nc.declare_dram_parameter("bwb", [128, 256], bf16, isOutput=False)
    y = nc.declare_dram_parameter("y", [rows_total, COLS], f32, isOutput=True)

    with (
        nc.sbuf_tensor("bw_sb", [128, 128], f32r) as bw,
        nc.sbuf_tensor("bwb_sb", [128, 256], bf16) as bwb,
        nc.sbuf_tensor("xin", [128, B_IN * 512], f32r) as xin,
        nc.sbuf_tensor("tq", [128, B_MID * 512], f32) as tq,
        nc.sbuf_tensor("tt", [128, B_MID * 512], bf16) as tt,
        nc.sbuf_tensor("oo", [128, B_OUT * 512], f32) as oo,
        nc.psum_tensor("zz", [128, N_PSUM * 512], f32) as zz,
        nc.psum_tensor("ww", [128, N_PSUM * 512], f32) as ww,
        nc.semaphore("load_sem") as load_sem,
        nc.semaphore("store_sem") as store_sem,
        nc.semaphore("pool_sem") as pool_sem,
        nc.semaphore("wt_sem") as wt_sem,
        nc.semaphore("pe1_sem") as pe1_sem,
        nc.semaphore("pe2_sem") as pe2_sem,
        nc.semaphore("dve1_sem") as dve1_sem,
        nc.semaphore("dve2_sem") as dve2_sem,
        nc.Block() as block,
    ):
        def slot(buf, i, n=N_PSUM):
            return buf[:, (i % n) * 512:(i % n + 1) * 512]

        def pair(buf, k, n=N_PSUM):
            # tiles (2k, 2k+1) -> contiguous [128, 1024] view
            return buf[:, (2 * k % n) * 512:(2 * k % n + 2) * 512]

        def sub32(ap, k):
            # [128, 512] slot -> [(p), (r: stride 128, 4), (c: 1, 32)] at
            # 32-column subblock k of each 128-run
            return ap.rearrange("p (r k c) -> p r k c", k=4, c=32)[:, :, k, :]

        @block.sync
        def _(sync):
            for q in range(n_iters // 2):
                src = x[2 * q * R_TILE:(2 * q + 2) * R_TILE, :].rearrange(
                    "(r rg) (ph inner) -> (rg ph) r inner", rg=4, inner=128
                )
                ld = sync.dma_start(
                    pair(xin, q, B_IN).rearrange(
                        "p (r inner) -> p r inner", inner=128
                    ),
                    src,
                )
                if 2 * q >= B_IN:
                    ld.wait_op(pe1_sem, 2 * q - B_IN + 2, "sem-ge")
                ld.then_inc(load_sem, 16)

        @block.tensor
        def _(tensor):
            tensor.wait_ge(wt_sem, 32)  # both weight DMAs done

            def mm1(i):
                if i >= N_PSUM:
                    # slack wait (T1 runs ahead): standalone, normally already
                    # satisfied so it doesn't hold the SEQ
                    tensor.wait_ge(dve1_sem, (i - N_PSUM) // 2 + 1)
                mm = tensor.matmul(
                    out=slot(zz, i),
                    lhsT=bw[:],
                    rhs=slot(xin, i, B_IN),
                    start=True,
                    stop=True,
                )
                mm.wait_op(load_sem, 16 * (i // 2 + 1), "sem-ge")
                mm.then_inc(pe1_sem)

            def mm2(j):
                if j >= N_PSUM:
                    tensor.wait_ge(dve2_sem, (j - N_PSUM) // 2 + 1)  # slack
                for i1m in range(4):
                    for j1m in range(4):
                        neg = bin(i1m & j1m).count("1") % 2
                        lhs = bwb[:, 128:256] if neg else bwb[:, 0:128]
                        mm = tensor.matmul(
                            out=sub32(slot(ww, j), i1m),
                            lhsT=lhs,
                            rhs=sub32(slot(tt, j, B_MID), j1m),
                            start=(j1m == 0),
                            stop=(j1m == 3),
                        )
                        if i1m == 0 and j1m == 0:
                            mm.wait_op(pool_sem, j // 2 + 1, "sem-ge")
                        if i1m == 3 and j1m == 3:
                            mm.then_inc(pe2_sem)

            for i in range(min(LA, n_iters)):
                mm1(i)
            for j in range(n_iters):
                if j + LA < n_iters:
                    mm1(j + LA)
                mm2(j)

        @block.gpsimd
        def _(gpsimd):
            # weight loads via SWDGE so they never contend with the first
            # data loads for the shared HWDGE slot
            gpsimd.dma_start(bw[:], bw_d[:]).then_inc(wt_sem, 16)
            gpsimd.dma_start(bwb[:], bwb_d[:]).then_inc(wt_sem, 16)
            # f32 -> bf16 rounding copy (SBUF->SBUF, pair-batched) on the
            # otherwise-idle Pool engine; GPSIMD cannot access PSUM, so T1
            # stages Z into tq first.
            for c in range(n_iters // 2):
                if 2 * c >= B_MID:
                    gpsimd.wait_ge(pe2_sem, 2 * c - B_MID + 2)  # tt free
                cp = gpsimd.tensor_copy(pair(tt, c, B_MID), pair(tq, c, B_MID))
                cp.wait_op(dve1_sem, c + 1, "sem-ge")
                cp.then_inc(pool_sem)

        @block.scalar
        def _(scalar):
            for k in range(n_iters // 2):
                dst = y[2 * k * R_TILE:(2 * k + 2) * R_TILE, :].rearrange(
                    "(r rg) (ph inner) -> (rg ph) r inner", rg=4, inner=128
                )
                st = scalar.dma_start(
                    dst,
                    pair(oo, k, B_OUT).rearrange(
                        "p (r inner) -> p r inner", inner=128
                    ),
                )
                st.wait_op(dve2_sem, k + 1, "sem-ge")
                st.then_inc(store_sem, 16)

        @block.vector
        def _(vector):
            n_pairs = n_iters // 2

            def t1(k):
                if 2 * k >= B_MID:
                    # tq slot free: conv of B_MID//2 pairs ago done
                    vector.wait_ge(pool_sem, k - B_MID // 2 + 1)
                tr = vector.transpose(pair(tq, k, B_MID), pair(zz, k))
                tr.wait_op(pe1_sem, 2 * k + 2, "sem-ge")
                tr.then_inc(dve1_sem)

            def t2(k):
                if 2 * k >= B_OUT:
                    vector.wait_ge(store_sem, 16 * (k - B_OUT // 2 + 1))  # slack
                tr = vector.transpose(pair(oo, k, B_OUT), pair(ww, k))
                tr.wait_op(pe2_sem, 2 * k + 2, "sem-ge")
                tr.then_inc(dve2_sem)

            for k in range(min(3, n_pairs)):
                t1(k)
            for k in range(n_pairs):
                if k + 3 < n_pairs:
                    t1(k + 3)
                t2(k)


    return nc


_CACHE = {}


def kernel(x: np.ndarray) -> np.ndarray:
    from concourse.bass_utils import run_bass_kernel_spmd

    assert x.shape == (ROWS, COLS) and x.dtype == np.float32

    if "nc" not in _CACHE:
        _CACHE["nc"] = _build_nc()
    nc = _CACHE["nc"]

    bw, bwb = _weights()

    core_ids = list(range(N_CORES))
    in_maps = [
        {
            "x": np.ascontiguousarray(x[i * ROWS_PER_CORE:(i + 1) * ROWS_PER_CORE]),
            "bw": bw,
            "bwb": bwb,
        }
        for i in core_ids
    ]
    res = run_bass_kernel_spmd(nc, in_maps, core_ids)
    out = np.empty((ROWS, COLS), dtype=np.float32)
    for i in core_ids:
        out[i * ROWS_PER_CORE:(i + 1) * ROWS_PER_CORE] = res.results[i]["y"]
    return out


# revision 31
# speedup vs baseline: 1.0058x; 1.0032x over previous
"""FWHT kernel for Trainium2: y = FWHT(x) along last axis.

x: (8192, 4096) fp32. Sharded row-wise (data-parallel) across 8 NeuronCores.

Math: FWHT (natural order) is y[i] = sum_j (-1)^{<i,j>} x[j] over 12-bit
indices, which factorizes over any aligned bit split. Split j = (j1h:5 |
j1m:2 | j0:5) and i likewise:
  y[i1h,i1m,i0] = sum H32[j1h,i1h] H4[j1m,i1m] H32[j0,i0] x[j1h,j1m,j0]

Layout is chosen so every DMA descriptor is a 512B contiguous run (the DMA
cost model charges 2x below 512B). Rows are interleaved 4-way into
partition groups: partition p = rg*32 + k, with rg = row mod 4 inside a
16-row tile (row_local = 4r + rg, r in [0,4)). The HBM access pattern
[(128,128),(16384,4),(1,128)] then has 128-element (512B) contiguous runs.

Per 16-row tile (free dim 512):
  SP     : DMA load  X[p=(rg,j1h), f=(r,j1m,j0)] as f32r    (512B runs)
  TENSOR : MM1  Z = B^T X, B = I4 (x) H32, contract j1h -> PSUM zz (f32r;
           DRAM x is declared f32r so the BIR verifier accepts DMA->matmul)
  VECTOR : T1   32x32 block transpose zz -> tq[p=(rg,j0)] SBUF f32
  POOL   : convert tq f32 -> tt bf16 (GPSIMD is the rounding producer for
           MM2's inputs; it cannot read PSUM, hence the tq staging)
  TENSOR : MM2  16 matmuls: W[.,(r,i1m,i1h)] += H4[j1m,i1m]*(B^T T) (bf16)
  VECTOR : T2   block transpose -> O[p=(rg,i1h), f=(r,i1m,i0)] SBUF f32
  ACT    : DMA store O -> y                                  (512B runs)

Scheduling: all DMAs move tile PAIRS (32 rows, 1024 descriptors) to halve
pressure on the shared HWDGE slot; T1/T2/conv are also pair-batched to
amortize access latency.  Waits are ATTACHED to instructions (one slot per
instruction, resolved in the engine wait queue) so they never block the
SEQ; only slack slot-recycling waits are standalone.  The critical cycle
T1 -> conv -> MM2 -> T2 -> (DVE in-order) -> T1 spans ~6.5us, so T1 runs
3 pairs ahead of T2 in the DVE stream (and MM1 6 tiles ahead on PE),
bringing the per-pair cost under the 2912 ns DMA cadence.  Weight loads
go through Pool/SWDGE so they never contend with data loads for HWDGE.

Per-core DMA: 32 MB at 512B/desc = 93.5 us, and the DMA device simulates
100% busy end-to-end; per-pair engine busy: DMA 2912 ns > DVE ~2380 >
PE ~2130 > Pool ~1520 > ACT ~690.  TimelineSim: 96527 ns/core vs the
195236 ns baseline (which paid the 2x sub-512B descriptor penalty):
2079 ns fill (barrier + DGE issue latency) + 93548 ns transfers (zero
idle) + 900 ns final DMA sem propagation (walrus requires every DMA to
carry a sem update, so this is irreducible).

Precision: x in f32r (~1e-4 rel), T in bf16 (~4e-3), weights are +-1
(exact); PSUM accumulation f32. Well within the 2e-2 tolerance.
"""

import numpy as np

N_CORES = 8
ROWS = 8192
COLS = 4096
ROWS_PER_CORE = ROWS // N_CORES  # 1024
R_TILE = 16                      # rows per tile -> free dim 512
N_ITERS = ROWS_PER_CORE // R_TILE  # 64

B_IN = 16   # xin slots
B_MID = 8   # tq/tt slots (pair-aligned)
B_OUT = 12  # oo slots (pair-aligned)
N_PSUM = 4  # zz/ww slots (4 banks each)
LA = 6      # MM1 lookahead (tiles) over the MM2 stream


def _sylvester(n: int) -> np.ndarray:
    H = np.array([[1.0]], dtype=np.float32)
    while H.shape[0] < n:
        H = np.block([[H, H], [H, -H]])
    return H.astype(np.float32)


def _weights():
    import ml_dtypes

    seed = np.tile(_sylvester(32), (4, 1)).astype(ml_dtypes.bfloat16)
    return np.ascontiguousarray(seed)


def _build_nc(n_iters: int = N_ITERS):
    import concourse.bass as bass
    import concourse.mybir as mybir

    assert n_iters % 2 == 0
    f32 = mybir.dt.float32
    f32r = mybir.dt.float32r
    bf16 = mybir.dt.bfloat16

    # Skip Bass.__init__'s const-AP Memsets on Pool: nothing in this kernel
    # reads the const tensors (no activation-bias users), and they make Pool
    # the laggard of the startup all_engine_barrier, delaying the first load
    # DMA by ~0.5us.
    _orig_memset = bass.BassGpSimd.memset
    bass.BassGpSimd.memset = lambda self, ap, value: None
    try:
        # detect_race_conditions=False: waits use sum semantics (each DMA adds
        # exactly +16 split across SDMA engines, per-engine FIFO), so >= 16*k
        # implies the first k DMAs completed.
        nc = bass.Bass(detect_race_conditions=False)
    finally:
        bass.BassGpSimd.memset = _orig_memset
    rows_total = n_iters * R_TILE
    x = nc.declare_dram_parameter("x", [rows_total, COLS], f32r, isOutput=False)
    seed_d = nc.declare_dram_parameter("hseed", [128, 32], bf16, isOutput=False)
    y = nc.declare_dram_parameter("y", [rows_total, COLS], f32, isOutput=True)

    with (
        nc.sbuf_tensor("bw_sb", [128, 128], f32r) as bw,
        nc.sbuf_tensor("bwb_sb", [128, 256], bf16) as bwb,
        nc.sbuf_tensor("hseed_sb", [128, 32], bf16) as sd,
        nc.sbuf_tensor("xin", [128, B_IN * 512], f32r) as xin,
        nc.sbuf_tensor("tq", [128, B_MID * 512], f32) as tq,
        nc.sbuf_tensor("tt", [128, B_MID * 512], bf16) as tt,
        nc.sbuf_tensor("oo", [128, B_OUT * 512], f32) as oo,
        nc.psum_tensor("zz", [128, N_PSUM * 512], f32) as zz,
        nc.psum_tensor("ww", [128, N_PSUM * 512], f32) as ww,
        nc.semaphore("load_sem") as load_sem,
        nc.semaphore("store_sem") as store_sem,
        nc.semaphore("pool_sem") as pool_sem,
        nc.semaphore("wt_sem") as wt_sem,
        nc.semaphore("pe1_sem") as pe1_sem,
        nc.semaphore("pe2_sem") as pe2_sem,
        nc.semaphore("dve1_sem") as dve1_sem,
        nc.semaphore("dve2_sem") as dve2_sem,
        nc.Block() as block,
    ):
        def slot(buf, i, n=N_PSUM):
            return buf[:, (i % n) * 512:(i % n + 1) * 512]

        def pair(buf, k, n=N_PSUM):
            # tiles (2k, 2k+1) -> contiguous [128, 1024] view
            return buf[:, (2 * k % n) * 512:(2 * k % n + 2) * 512]

        def sub32(ap, k):
            # [128, 512] slot -> [(p), (r: stride 128, 4), (c: 1, 32)] at
            # 32-column subblock k of each 128-run
            return ap.rearrange("p (r k c) -> p r k c", k=4, c=32)[:, :, k, :]

        @block.sync
        def _(sync):
            for q in range(n_iters // 2):
                src = x[2 * q * R_TILE:(2 * q + 2) * R_TILE, :].rearrange(
                    "(r rg) (ph inner) -> (rg ph) r inner", rg=4, inner=128
                )
                ld = sync.dma_start(
                    pair(xin, q, B_IN).rearrange(
                        "p (r inner) -> p r inner", inner=128
                    ),
                    src,
                )
                if 2 * q >= B_IN:
                    ld.wait_op(pe1_sem, 2 * q - B_IN + 2, "sem-ge")
                ld.then_inc(load_sem, 16)

        @block.tensor
        def _(tensor):
            tensor.wait_ge(wt_sem, 17)  # weights built on-chip

            def mm1(i):
                if i >= N_PSUM:
                    # slack wait (T1 runs ahead): standalone, normally already
                    # satisfied so it doesn't hold the SEQ
                    tensor.wait_ge(dve1_sem, (i - N_PSUM) // 2 + 1)
                mm = tensor.matmul(
                    out=slot(zz, i),
                    lhsT=bw[:],
                    rhs=slot(xin, i, B_IN),
                    start=True,
                    stop=True,
                )
                mm.wait_op(load_sem, 16 * (i // 2 + 1), "sem-ge")
                mm.then_inc(pe1_sem)

            def mm2(j):
                if j >= N_PSUM:
                    tensor.wait_ge(dve2_sem, (j - N_PSUM) // 2 + 1)  # slack
                for i1m in range(4):
                    for j1m in range(4):
                        neg = bin(i1m & j1m).count("1") % 2
                        lhs = bwb[:, 128:256] if neg else bwb[:, 0:128]
                        mm = tensor.matmul(
                            out=sub32(slot(ww, j), i1m),
                            lhsT=lhs,
                            rhs=sub32(slot(tt, j, B_MID), j1m),
                            start=(j1m == 0),
                            stop=(j1m == 3),
                        )
                        if i1m == 0 and j1m == 0:
                            mm.wait_op(pool_sem, j // 2 + 1, "sem-ge")
                        if i1m == 3 and j1m == 3:
                            mm.then_inc(pe2_sem)

            for i in range(min(LA, n_iters)):
                mm1(i)
            for j in range(n_iters):
                if j + LA < n_iters:
                    mm1(j + LA)
                mm2(j)

        @block.gpsimd
        def _(gpsimd):
            # Build B = I4 (x) H32 on-chip from a 2KB seed (H32 tiled across
            # the four 32-partition groups) instead of DMAing 128KB of
            # weights: the seed transfer costs ~56ns on the contended DMA
            # device vs 364ns, and all construction runs on the idle Pool
            # engine during the pipeline fill. GPSIMD TensorCopy is a
            # rounding-capable producer, satisfying the FP32r matmul input
            # rule for bw. tt[:, 0:128] serves as a zero scratch; the first
            # conv overwrites it only later (Pool is in-order).
            gpsimd.dma_start(sd[:], seed_d[:]).then_inc(wt_sem, 16)
            zs = tt[:, 0:128]
            gpsimd.memset(zs, 0.0)
            gpsimd.wait_ge(wt_sem, 16)  # seed landed
            gpsimd.tensor_copy(bw[:], zs)
            for g in range(4):
                gpsimd.tensor_copy(
                    bw[g * 32:(g + 1) * 32, g * 32:g * 32 + 32],
                    sd[g * 32:(g + 1) * 32, :],
                )
            gpsimd.tensor_copy(bwb[:, 0:128], zs)
            gpsimd.tensor_copy(bwb[:, 128:256], zs)
            for g in range(4):
                gpsimd.tensor_copy(
                    bwb[g * 32:(g + 1) * 32, g * 32:g * 32 + 32],
                    sd[g * 32:(g + 1) * 32, :],
                )
                ts = gpsimd.tensor_scalar_mul(
                    bwb[g * 32:(g + 1) * 32, 128 + g * 32:128 + g * 32 + 32],
                    sd[g * 32:(g + 1) * 32, :],
                    -1.0,
                )
                if g == 3:
                    ts.then_inc(wt_sem)  # wt_sem = 17: weights ready
            # f32 -> bf16 rounding copy (SBUF->SBUF, pair-batched) on the
            # otherwise-idle Pool engine; GPSIMD cannot access PSUM, so T1
            # stages Z into tq first.
            for c in range(n_iters // 2):
                if 2 * c >= B_MID:
                    gpsimd.wait_ge(pe2_sem, 2 * c - B_MID + 2)  # tt free
                cp = gpsimd.tensor_copy(pair(tt, c, B_MID), pair(tq, c, B_MID))
                cp.wait_op(dve1_sem, c + 1, "sem-ge")
                cp.then_inc(pool_sem)

        @block.scalar
        def _(scalar):
            for k in range(n_iters // 2):
                dst = y[2 * k * R_TILE:(2 * k + 2) * R_TILE, :].rearrange(
                    "(r rg) (ph inner) -> (rg ph) r inner", rg=4, inner=128
                )
                st = scalar.dma_start(
                    dst,
                    pair(oo, k, B_OUT).rearrange(
                        "p (r inner) -> p r inner", inner=128
                    ),
                )
                st.wait_op(dve2_sem, k + 1, "sem-ge")
                st.then_inc(store_sem, 16)

        @block.vector
        def _(vector):
            n_pairs = n_iters // 2

            def t1(k):
                if 2 * k >= B_MID:
                    # tq slot free: conv of B_MID//2 pairs ago done
                    vector.wait_ge(pool_sem, k - B_MID // 2 + 1)
                tr = vector.transpose(pair(tq, k, B_MID), pair(zz, k))
                tr.wait_op(pe1_sem, 2 * k + 2, "sem-ge")
                tr.then_inc(dve1_sem)

            def t2(k):
                if 2 * k >= B_OUT:
                    vector.wait_ge(store_sem, 16 * (k - B_OUT // 2 + 1))  # slack
                tr = vector.transpose(pair(oo, k, B_OUT), pair(ww, k))
                tr.wait_op(pe2_sem, 2 * k + 2, "sem-ge")
                tr.then_inc(dve2_sem)

            for k in range(min(3, n_pairs)):
                t1(k)
            for k in range(n_pairs):
                if k + 3 < n_pairs:
                    t1(k + 3)
                t2(k)


    return nc


_CACHE = {}


def kernel(x: np.ndarray) -> np.ndarray:
    from concourse.bass_utils import run_bass_kernel_spmd

    assert x.shape == (ROWS, COLS) and x.dtype == np.float32

    if "nc" not in _CACHE:
        _CACHE["nc"] = _build_nc()
    nc = _CACHE["nc"]

    seed = _weights()

    core_ids = list(range(N_CORES))
    in_maps = [
        {
            "x": np.ascontiguousarray(x[i * ROWS_PER_CORE:(i + 1) * ROWS_PER_CORE]),
            "hseed": seed,
        }
        for i in core_ids
    ]
    res = run_bass_kernel_spmd(nc, in_maps, core_ids)
    out = np.empty((ROWS, COLS), dtype=np.float32)
    for i in core_ids:
        out[i * ROWS_PER_CORE:(i + 1) * ROWS_PER_CORE] = res.results[i]["y"]
    return out


# revision 34
# speedup vs baseline: 1.0106x; 1.0048x over previous
"""FWHT kernel for Trainium2: y = FWHT(x) along last axis.

x: (8192, 4096) fp32. Sharded row-wise (data-parallel) across 8 NeuronCores.

Math: FWHT (natural order) is y[i] = sum_j (-1)^{<i,j>} x[j] over 12-bit
indices, which factorizes over any aligned bit split. Split j = (j1h:5 |
j1m:2 | j0:5) and i likewise:
  y[i1h,i1m,i0] = sum H32[j1h,i1h] H4[j1m,i1m] H32[j0,i0] x[j1h,j1m,j0]

Layout is chosen so every DMA descriptor is a 512B contiguous run (the DMA
cost model charges 2x below 512B). Rows are interleaved 4-way into
partition groups: partition p = rg*32 + k, with rg = row mod 4 inside a
16-row tile (row_local = 4r + rg, r in [0,4)). The HBM access pattern
[(128,128),(16384,4),(1,128)] then has 128-element (512B) contiguous runs.

Per 16-row tile (free dim 512):
  SP     : DMA load  X[p=(rg,j1h), f=(r,j1m,j0)] as f32r    (512B runs)
  TENSOR : MM1  Z = B^T X, B = I4 (x) H32, contract j1h -> PSUM zz (f32r;
           DRAM x is declared f32r so the BIR verifier accepts DMA->matmul)
  VECTOR : T1   32x32 block transpose zz -> tq[p=(rg,j0)] SBUF f32
  POOL   : convert tq f32 -> tt bf16 (GPSIMD is the rounding producer for
           MM2's inputs; it cannot read PSUM, hence the tq staging)
  TENSOR : MM2  16 matmuls: W[.,(r,i1m,i1h)] += H4[j1m,i1m]*(B^T T) (bf16)
  VECTOR : T2   block transpose -> O[p=(rg,i1h), f=(r,i1m,i0)] SBUF f32
  ACT    : DMA store O -> y                                  (512B runs)

Scheduling: all DMAs move tile PAIRS (32 rows, 1024 descriptors) to halve
pressure on the shared HWDGE slot; T1/T2/conv are also pair-batched to
amortize access latency.  Waits are ATTACHED to instructions (one slot per
instruction, resolved in the engine wait queue) so they never block the
SEQ; only slack slot-recycling waits are standalone.  The critical cycle
T1 -> conv -> MM2 -> T2 -> (DVE in-order) -> T1 spans ~6.5us, so T1 runs
3 pairs ahead of T2 in the DVE stream (and MM1 6 tiles ahead on PE),
bringing the per-pair cost under the 2912 ns DMA cadence.  Weights are
built on-chip from a 2KB seed (Pool/SWDGE) instead of DMAing 128KB.

Per-core DMA: 32 MB at 512B/desc = 93.2 us, and the DMA device simulates
100% busy end-to-end; per-pair engine busy: DMA 2912 ns > DVE ~2380 >
PE ~2130 > Pool ~1520 > ACT ~690.  TimelineSim: 96219 ns/core vs the
195236 ns baseline (which paid the 2x sub-512B descriptor penalty):
2079 ns fill (barrier + DGE issue latency, at its minimum) + 93240 ns
transfers (93184 data + 56 seed, zero idle) + 900 ns final DMA sem
propagation (walrus requires every DMA to carry a sem update, so this
tail is irreducible).

Precision: x in f32r (~1e-4 rel), T in bf16 (~4e-3), weights are +-1
(exact); PSUM accumulation f32. Well within the 2e-2 tolerance.
"""

import numpy as np

N_CORES = 8
ROWS = 8192
COLS = 4096
ROWS_PER_CORE = ROWS // N_CORES  # 1024
R_TILE = 16                      # rows per tile -> free dim 512
N_ITERS = ROWS_PER_CORE // R_TILE  # 64

B_IN = 16   # xin slots
B_MID = 8   # tq/tt slots (pair-aligned)
B_OUT = 12  # oo slots (pair-aligned)
N_PSUM = 4  # zz/ww slots (4 banks each)
LA = 6      # MM1 lookahead (tiles) over the MM2 stream


def _sylvester(n: int) -> np.ndarray:
    H = np.array([[1.0]], dtype=np.float32)
    while H.shape[0] < n:
        H = np.block([[H, H], [H, -H]])
    return H.astype(np.float32)


def _weights():
    import ml_dtypes

    seed = np.tile(_sylvester(32), (4, 1)).astype(ml_dtypes.bfloat16)
    return np.ascontiguousarray(seed)


def _build_nc(n_iters: int = N_ITERS):
    import concourse.bass as bass
    import concourse.mybir as mybir

    assert n_iters % 2 == 0
    f32 = mybir.dt.float32
    f32r = mybir.dt.float32r
    bf16 = mybir.dt.bfloat16

    # Skip Bass.__init__'s const-AP Memsets on Pool: nothing in this kernel
    # reads the const tensors (no activation-bias users), and they make Pool
    # the laggard of the startup all_engine_barrier, delaying the first load
    # DMA by ~0.5us.
    _orig_memset = bass.BassGpSimd.memset
    bass.BassGpSimd.memset = lambda self, ap, value: None
    _patched = []
    if "preamble" not in bass.BassEngine.__dict__:
        bass.BassEngine.preamble = lambda self: None
        _patched.append(bass.BassEngine)
    try:
        # detect_race_conditions=False: waits use sum semantics (each DMA adds
        # exactly +16 split across SDMA engines, per-engine FIFO), so >= 16*k
        # implies the first k DMAs completed.
        nc = bass.Bass(detect_race_conditions=False)
    finally:
        bass.BassGpSimd.memset = _orig_memset
        for _cls in _patched:
            del _cls.preamble  # restore the rust preamble
    rows_total = n_iters * R_TILE
    x = nc.declare_dram_parameter("x", [rows_total, COLS], f32r, isOutput=False)
    seed_d = nc.declare_dram_parameter("hseed", [128, 32], bf16, isOutput=False)
    y = nc.declare_dram_parameter("y", [rows_total, COLS], f32, isOutput=True)

    with (
        nc.sbuf_tensor("bw_sb", [128, 128], f32r) as bw,
        nc.sbuf_tensor("bwb_sb", [128, 256], bf16) as bwb,
        nc.sbuf_tensor("hseed_sb", [128, 32], bf16) as sd,
        nc.sbuf_tensor("xin", [128, B_IN * 512], f32r) as xin,
        nc.sbuf_tensor("tq", [128, B_MID * 512], f32) as tq,
        nc.sbuf_tensor("tt", [128, B_MID * 512], bf16) as tt,
        nc.sbuf_tensor("oo", [128, B_OUT * 512], f32) as oo,
        nc.psum_tensor("zz", [128, N_PSUM * 512], f32) as zz,
        nc.psum_tensor("ww", [128, N_PSUM * 512], f32) as ww,
        nc.semaphore("load_sem") as load_sem,
        nc.semaphore("store_sem") as store_sem,
        nc.semaphore("pool_sem") as pool_sem,
        nc.semaphore("wt_sem") as wt_sem,
        nc.semaphore("pe1_sem") as pe1_sem,
        nc.semaphore("pe2_sem") as pe2_sem,
        nc.semaphore("dve1_sem") as dve1_sem,
        nc.semaphore("dve2_sem") as dve2_sem,
        nc.Block() as block,
    ):
        def slot(buf, i, n=N_PSUM):
            return buf[:, (i % n) * 512:(i % n + 1) * 512]

        def pair(buf, k, n=N_PSUM):
            # tiles (2k, 2k+1) -> contiguous [128, 1024] view
            return buf[:, (2 * k % n) * 512:(2 * k % n + 2) * 512]

        def sub32(ap, k):
            # [128, 512] slot -> [(p), (r: stride 128, 4), (c: 1, 32)] at
            # 32-column subblock k of each 128-run
            return ap.rearrange("p (r k c) -> p r k c", k=4, c=32)[:, :, k, :]

        @block.sync
        def _(sync):
            for q in range(n_iters // 2):
                src = x[2 * q * R_TILE:(2 * q + 2) * R_TILE, :].rearrange(
                    "(r rg) (ph inner) -> (rg ph) r inner", rg=4, inner=128
                )
                ld = sync.dma_start(
                    pair(xin, q, B_IN).rearrange(
                        "p (r inner) -> p r inner", inner=128
                    ),
                    src,
                )
                if 2 * q >= B_IN:
                    ld.wait_op(pe1_sem, 2 * q - B_IN + 2, "sem-ge")
                ld.then_inc(load_sem, 16)

        @block.tensor
        def _(tensor):
            tensor.wait_ge(wt_sem, 17)  # weights built on-chip

            def mm1(i):
                if i >= N_PSUM:
                    # slack wait (T1 runs ahead): standalone, normally already
                    # satisfied so it doesn't hold the SEQ
                    tensor.wait_ge(dve1_sem, (i - N_PSUM) // 2 + 1)
                mm = tensor.matmul(
                    out=slot(zz, i),
                    lhsT=bw[:],
                    rhs=slot(xin, i, B_IN),
                    start=True,
                    stop=True,
                )
                mm.wait_op(load_sem, 16 * (i // 2 + 1), "sem-ge")
                mm.then_inc(pe1_sem)

            def mm2(j):
                if j >= N_PSUM:
                    tensor.wait_ge(dve2_sem, (j - N_PSUM) // 2 + 1)  # slack
                for i1m in range(4):
                    for j1m in range(4):
                        neg = bin(i1m & j1m).count("1") % 2
                        lhs = bwb[:, 128:256] if neg else bwb[:, 0:128]
                        mm = tensor.matmul(
                            out=sub32(slot(ww, j), i1m),
                            lhsT=lhs,
                            rhs=sub32(slot(tt, j, B_MID), j1m),
                            start=(j1m == 0),
                            stop=(j1m == 3),
                        )
                        if i1m == 0 and j1m == 0:
                            mm.wait_op(pool_sem, j // 2 + 1, "sem-ge")
                        if i1m == 3 and j1m == 3:
                            mm.then_inc(pe2_sem)

            for i in range(min(LA, n_iters)):
                mm1(i)
            for j in range(n_iters):
                if j + LA < n_iters:
                    mm1(j + LA)
                mm2(j)

        @block.gpsimd
        def _(gpsimd):
            # Build B = I4 (x) H32 on-chip from a 2KB seed (H32 tiled across
            # the four 32-partition groups) instead of DMAing 128KB of
            # weights: the seed transfer costs ~56ns on the contended DMA
            # device vs 364ns, and all construction runs on the idle Pool
            # engine during the pipeline fill. GPSIMD TensorCopy is a
            # rounding-capable producer, satisfying the FP32r matmul input
            # rule for bw. tt[:, 0:128] serves as a zero scratch; the first
            # conv overwrites it only later (Pool is in-order).
            gpsimd.dma_start(sd[:], seed_d[:]).then_inc(wt_sem, 16)
            zs = tt[:, 0:128]
            gpsimd.memset(zs, 0.0)
            gpsimd.wait_ge(wt_sem, 16)  # seed landed
            gpsimd.tensor_copy(bw[:], zs)
            for g in range(4):
                gpsimd.tensor_copy(
                    bw[g * 32:(g + 1) * 32, g * 32:g * 32 + 32],
                    sd[g * 32:(g + 1) * 32, :],
                )
            gpsimd.tensor_copy(bwb[:, 0:128], zs)
            gpsimd.tensor_copy(bwb[:, 128:256], zs)
            for g in range(4):
                gpsimd.tensor_copy(
                    bwb[g * 32:(g + 1) * 32, g * 32:g * 32 + 32],
                    sd[g * 32:(g + 1) * 32, :],
                )
                ts = gpsimd.tensor_scalar_mul(
                    bwb[g * 32:(g + 1) * 32, 128 + g * 32:128 + g * 32 + 32],
                    sd[g * 32:(g + 1) * 32, :],
                    -1.0,
                )
                if g == 3:
                    ts.then_inc(wt_sem)  # wt_sem = 17: weights ready
            # f32 -> bf16 rounding copy (SBUF->SBUF, pair-batched) on the
            # otherwise-idle Pool engine; GPSIMD cannot access PSUM, so T1
            # stages Z into tq first.
            for c in range(n_iters // 2):
                if 2 * c >= B_MID:
                    gpsimd.wait_ge(pe2_sem, 2 * c - B_MID + 2)  # tt free
                cp = gpsimd.tensor_copy(pair(tt, c, B_MID), pair(tq, c, B_MID))
                cp.wait_op(dve1_sem, c + 1, "sem-ge")
                cp.then_inc(pool_sem)

        @block.scalar
        def _(scalar):
            for k in range(n_iters // 2):
                dst = y[2 * k * R_TILE:(2 * k + 2) * R_TILE, :].rearrange(
                    "(r rg) (ph inner) -> (rg ph) r inner", rg=4, inner=128
                )
                st = scalar.dma_start(
                    dst,
                    pair(oo, k, B_OUT).rearrange(
                        "p (r inner) -> p r inner", inner=128
                    ),
                )
                st.wait_op(dve2_sem, k + 1, "sem-ge")
                st.then_inc(store_sem, 16)

        @block.vector
        def _(vector):
            n_pairs = n_iters // 2

            def t1(k):
                if 2 * k >= B_MID:
                    # tq slot free: conv of B_MID//2 pairs ago done
                    vector.wait_ge(pool_sem, k - B_MID // 2 + 1)
                tr = vector.transpose(pair(tq, k, B_MID), pair(zz, k))
                tr.wait_op(pe1_sem, 2 * k + 2, "sem-ge")
                tr.then_inc(dve1_sem)

            def t2(k):
                if 2 * k >= B_OUT:
                    vector.wait_ge(store_sem, 16 * (k - B_OUT // 2 + 1))  # slack
                tr = vector.transpose(pair(oo, k, B_OUT), pair(ww, k))
                tr.wait_op(pe2_sem, 2 * k + 2, "sem-ge")
                tr.then_inc(dve2_sem)

            for k in range(min(3, n_pairs)):
                t1(k)
            for k in range(n_pairs):
                if k + 3 < n_pairs:
                    t1(k + 3)
                t2(k)


    return nc


_CACHE = {}


def kernel(x: np.ndarray) -> np.ndarray:
    from concourse.bass_utils import run_bass_kernel_spmd

    assert x.shape == (ROWS, COLS) and x.dtype == np.float32

    if "nc" not in _CACHE:
        _CACHE["nc"] = _build_nc()
    nc = _CACHE["nc"]

    seed = _weights()

    core_ids = list(range(N_CORES))
    in_maps = [
        {
            "x": np.ascontiguousarray(x[i * ROWS_PER_CORE:(i + 1) * ROWS_PER_CORE]),
            "hseed": seed,
        }
        for i in core_ids
    ]
    res = run_bass_kernel_spmd(nc, in_maps, core_ids)
    out = np.empty((ROWS, COLS), dtype=np.float32)
    for i in core_ids:
        out[i * ROWS_PER_CORE:(i + 1) * ROWS_PER_CORE] = res.results[i]["y"]
    return out
